# Initial kernel scaffold
#
"""LinearWithLoRA on 8 TRN2 NeuronCores.

y = x @ W.T + b + 2.0 * (x @ A.T) @ B.T
  x: [4, 2048, 2048] f32, W: [2048, 2048], b: [2048], A: [16, 2048], B: [2048, 16]

Strategy: data-parallel over tokens (8192 tokens -> 1024 per core). Each core
reads its x shard + the full (replicated) W/A/B/b and computes its out shard;
no collectives. Host pre-transposes x and W so both matmul operands are
K(=d_in)-major in DRAM, which makes every DMA contiguous-ish and avoids any
on-device transposes.

Matmuls run in float32r (TF32-like, full PE rate for moving dim >= 256,
~6e-4 rel err at K=2048). The LoRA low-rank product lowT = (x @ A.T).T is
computed once per core ([16, 1024]) and its contribution accumulates into the
same PSUM banks as the base matmul. The bias is added in exact fp32 during
PSUM->SBUF eviction on the vector engine.
"""

import numpy as np

import concourse.bass as bass
import concourse.mybir as mybir
import concourse.tile as tile
from concourse import bacc
from concourse.bass import ds, ts
from concourse.bass_utils import run_bass_kernel_spmd

B, S, D_IN, D_OUT, R = 4, 2048, 2048, 2048, 16
SCALING = 32.0 / 16.0
N_CORES = 8
TOK = B * S  # 8192
TOK_SHARD = TOK // N_CORES  # 1024
P = 128
KO = D_IN // P  # 16 contraction tiles
N_CHUNK = 512  # psum bank limit for f32 moving operand
N_CHUNKS = D_OUT // N_CHUNK  # 4
M_TILES = TOK_SHARD // P  # 8
T_CHUNK = 512
T_CHUNKS = TOK_SHARD // T_CHUNK  # 2

_nc_cache = []


def _build():
    f32 = mybir.dt.float32
    f32r = mybir.dt.float32r

    nc = bacc.Bacc(None, target_bir_lowering=False)
    xT = nc.dram_tensor("xT", [D_IN, TOK_SHARD], f32r, kind="ExternalInput")
    wT = nc.dram_tensor("wT", [D_IN, D_OUT], f32r, kind="ExternalInput")
    aT = nc.dram_tensor("aT", [D_IN, R], f32r, kind="ExternalInput")
    bT = nc.dram_tensor("bT", [R, D_OUT], f32r, kind="ExternalInput")  # 2*lora_B.T
    bias = nc.dram_tensor("bias", [1, D_OUT], f32, kind="ExternalInput")
    out = nc.dram_tensor("out", [TOK_SHARD, D_OUT], f32, kind="ExternalOutput")

    xT3 = xT.rearrange("(ko p) t -> p ko t", p=P)
    wT3 = wT.rearrange("(ko p) n -> p ko n", p=P)
    aT3 = aT.rearrange("(ko p) r -> p ko r", p=P)

    with tile.TileContext(nc) as tc:
        with (
            tc.tile_pool(name="xpool", bufs=1) as xpool,
            tc.tile_pool(name="wpool", bufs=2) as wpool,
            tc.tile_pool(name="cpool", bufs=1) as cpool,
            tc.tile_pool(name="opool", bufs=4) as opool,
            tc.tile_pool(name="ppool", bufs=4, space="PSUM") as ppool,
            tc.tile_pool(name="lppool", bufs=2, space="PSUM") as lppool,
        ):
            # Constants / resident tensors
            at = cpool.tile([P, KO, R], f32r)
            nc.sync.dma_start(out=at[:], in_=aT3[:])
            bt = cpool.tile([R, D_OUT], f32r)
            nc.sync.dma_start(out=bt[:], in_=bT[:])
            bias_t = cpool.tile([1, D_OUT], f32)
            nc.sync.dma_start(out=bias_t[:], in_=bias[:])

            # x shard, fully resident: [128, 16, 1024] = 64 KB/partition.
            # Loaded in token chunks so the LoRA lowT matmuls can start early.
            xt = xpool.tile([P, KO, TOK_SHARD], f32r)
            for t in range(T_CHUNKS):
                nc.sync.dma_start(
                    out=xt[:, :, ts(t, T_CHUNK)], in_=xT3[:, :, ts(t, T_CHUNK)]
                )

            # lowT[r, tok] = A @ x.T, in f32r for the second LoRA matmul.
            lowt = cpool.tile([R, TOK_SHARD], f32r)
            for t in range(T_CHUNKS):
                pl = lppool.tile([R, T_CHUNK], f32)
                for k in range(KO):
                    nc.tensor.matmul(
                        pl[:],
                        at[:, k, :],
                        xt[:, k, ts(t, T_CHUNK)],
                        start=(k == 0),
                        stop=(k == KO - 1),
                    )
                nc.vector.tensor_copy(lowt[:, ts(t, T_CHUNK)], pl[:])

            # Main loop: stream W by dout chunk; x stays resident.
            for n in range(N_CHUNKS):
                wt = wpool.tile([P, KO, N_CHUNK], f32r, tag="w")
                nc.sync.dma_start(out=wt[:], in_=wT3[:, :, ts(n, N_CHUNK)])
                for m in range(M_TILES):
                    ps = ppool.tile([P, N_CHUNK], f32)
                    for k in range(KO):
                        nc.tensor.matmul(
                            ps[:],
                            xt[:, k, ts(m, P)],
                            wt[:, k, :],
                            start=(k == 0),
                            stop=False,
                        )
                    # LoRA contribution accumulates into the same bank.
                    nc.tensor.matmul(
                        ps[:],
                        lowt[:, ts(m, P)],
                        bt[:, ts(n, N_CHUNK)],
                        start=False,
                        stop=True,
                    )
                    ot = opool.tile([P, N_CHUNK], f32)
                    nc.vector.tensor_add(
                        ot[:],
                        ps[:],
                        bias_t[:1, ts(n, N_CHUNK)].partition_broadcast(P),
                    )
                    nc.sync.dma_start(
                        out=out[ts(m, P), ts(n, N_CHUNK)], in_=ot[:]
                    )

    nc.compile()
    return nc


def kernel(x, W, b, lora_A, lora_B):
    x = np.asarray(x, dtype=np.float32)
    W = np.asarray(W, dtype=np.float32)
    b = np.asarray(b, dtype=np.float32)
    lora_A = np.asarray(lora_A, dtype=np.float32)
    lora_B = np.asarray(lora_B, dtype=np.float32)

    xT = np.ascontiguousarray(x.reshape(TOK, D_IN).T)  # [D_IN, TOK]
    wT = np.ascontiguousarray(W.T)  # [D_IN, D_OUT]
    aT = np.ascontiguousarray(lora_A.T)  # [D_IN, R]
    bT = np.ascontiguousarray(SCALING * lora_B.T)  # [R, D_OUT]
    bias = np.ascontiguousarray(b[None, :])  # [1, D_OUT]

    if not _nc_cache:
        _nc_cache.append(_build())
    nc = _nc_cache[0]

    in_maps = [
        {
            "xT": np.ascontiguousarray(xT[:, i * TOK_SHARD : (i + 1) * TOK_SHARD]),
            "wT": wT,
            "aT": aT,
            "bT": bT,
            "bias": bias,
        }
        for i in range(N_CORES)
    ]
    res = run_bass_kernel_spmd(nc, in_maps, list(range(N_CORES)))
    out = np.concatenate([res.results[i]["out"] for i in range(N_CORES)], axis=0)
    return out.reshape(B, S, D_OUT)


# revision 5
# speedup vs baseline: 1.1801x; 1.1801x over previous
"""LinearWithLoRA on 8 TRN2 NeuronCores.

y = x @ W.T + b + 2.0 * (x @ A.T) @ B.T
  x: [4, 2048, 2048] f32, W: [2048, 2048], b: [2048], A: [16, 2048], B: [2048, 16]

Strategy: data-parallel over tokens (8192 tokens -> 1024 per core). Each core
reads its x shard + the full (replicated) W/A/B/b and computes its out shard;
no collectives. Host pre-transposes x and W so both matmul operands are
K(=d_in)-major in DRAM, which makes every DMA contiguous-ish and avoids any
on-device transposes.

Matmuls run in float32r (TF32-like, full PE rate for moving dim >= 256,
~1e-4 rel err at K=2048). The LoRA low-rank product lowT = (x @ A.T).T is
computed once per core ([16, 1024]) and its contribution accumulates into the
same PSUM banks as the base matmul. The bias is added in exact fp32 during
PSUM->SBUF eviction on the vector engine.
"""

import numpy as np

import concourse.bass as bass
import concourse.mybir as mybir
import concourse.tile as tile
from concourse import bacc
from concourse.bass import ds, ts
from concourse.bass_utils import run_bass_kernel_spmd

B, S, D_IN, D_OUT, R = 4, 2048, 2048, 2048, 16
SCALING = 32.0 / 16.0
N_CORES = 8
TOK = B * S  # 8192
TOK_SHARD = TOK // N_CORES  # 1024
P = 128
KO = D_IN // P  # 16 contraction tiles
N_CHUNK = 512  # psum bank limit for f32 moving operand
N_CHUNKS = D_OUT // N_CHUNK  # 4
M_TILES = TOK_SHARD // P  # 8
T_CHUNK = 512
T_CHUNKS = TOK_SHARD // T_CHUNK  # 2

_nc_cache = {}


def _build(reps=1, timing=False):
    f32 = mybir.dt.float32
    f32r = mybir.dt.float32r

    nc = bacc.Bacc(None, target_bir_lowering=False)
    xT = nc.dram_tensor("xT", [D_IN, TOK_SHARD], f32r, kind="ExternalInput")
    wT = nc.dram_tensor("wT", [D_IN, D_OUT], f32r, kind="ExternalInput")
    aT = nc.dram_tensor("aT", [D_IN, R], f32r, kind="ExternalInput")
    bT = nc.dram_tensor("bT", [R, D_OUT], f32r, kind="ExternalInput")  # 2*lora_B.T
    bias = nc.dram_tensor("bias", [1, D_OUT], f32, kind="ExternalInput")
    if timing:
        nc.dram_tensor("tiny_out", [1, 1], f32, kind="ExternalOutput")
        out = nc.dram_tensor("oscratch", [TOK_SHARD, D_OUT], f32)  # internal
    else:
        out = nc.dram_tensor("out", [TOK_SHARD, D_OUT], f32, kind="ExternalOutput")

    xT3 = xT.rearrange("(ko p) t -> p ko t", p=P)
    wT3 = wT.rearrange("(ko p) n -> p ko n", p=P)
    aT3 = aT.rearrange("(ko p) r -> p ko r", p=P)

    with tile.TileContext(nc) as tc:
        with (
            tc.tile_pool(name="xpool", bufs=1) as xpool,
            tc.tile_pool(name="wpool", bufs=2) as wpool,
            tc.tile_pool(name="cpool", bufs=1) as cpool,
            tc.tile_pool(name="opool", bufs=4) as opool,
            tc.tile_pool(name="ppool", bufs=4, space="PSUM") as ppool,
            tc.tile_pool(name="lppool", bufs=2, space="PSUM") as lppool,
        ):
            # Constants / resident tensors
            at = cpool.tile([P, KO, R], f32r)
            nc.sync.dma_start(out=at[:], in_=aT3[:])
            bt = cpool.tile([R, D_OUT], f32r)
            nc.sync.dma_start(out=bt[:], in_=bT[:])
            # Bias replicated to all partitions via broadcast DMA, so the
            # eviction add is a plain [P, N] tensor_tensor op.
            bias_t = cpool.tile([P, D_OUT], f32)
            nc.sync.dma_start(out=bias_t[:], in_=bias[:].to_broadcast((P, D_OUT)))

            # x shard, fully resident: [128, 16, 1024] = 64 KB/partition.
            # Loaded per k-tile so the k-ascending matmul streams can start
            # as soon as the first 512 KB lands.
            xt = xpool.tile([P, KO, TOK_SHARD], f32r)
            for k in range(KO):
                nc.sync.dma_start(out=xt[:, k, :], in_=xT3[:, k, :])

            # lowT[r, tok] = A @ x.T, in f32r for the second LoRA matmul.
            lowt = cpool.tile([R, TOK_SHARD], f32r)
            for t in range(T_CHUNKS):
                pl = lppool.tile([R, T_CHUNK], f32)
                for k in range(KO):
                    nc.tensor.matmul(
                        pl[:],
                        at[:, k, :],
                        xt[:, k, ts(t, T_CHUNK)],
                        start=(k == 0),
                        stop=(k == KO - 1),
                    )
                nc.vector.tensor_copy(lowt[:, ts(t, T_CHUNK)], pl[:])

            # Main loop: stream W by (dout chunk, k-group); x stays resident.
            for _ in range(reps):
                for n in range(N_CHUNKS):
                    wt = wpool.tile([P, KO, N_CHUNK], f32r, tag="w")
                    for kg in range(4):  # 4 k-groups of 4 -> 1 MiB DMAs
                        nc.sync.dma_start(
                            out=wt[:, ds(kg * 4, 4), :],
                            in_=wT3[:, ds(kg * 4, 4), ts(n, N_CHUNK)],
                        )
                    for m in range(M_TILES):
                        ps = ppool.tile([P, N_CHUNK], f32)
                        for k in range(KO):
                            nc.tensor.matmul(
                                ps[:],
                                xt[:, k, ts(m, P)],
                                wt[:, k, :],
                                start=(k == 0),
                                stop=False,
                            )
                        # LoRA contribution accumulates into the same bank.
                        nc.tensor.matmul(
                            ps[:],
                            lowt[:, ts(m, P)],
                            bt[:, ts(n, N_CHUNK)],
                            start=False,
                            stop=True,
                        )
                        ot = opool.tile([P, N_CHUNK], f32)
                        nc.vector.tensor_add(
                            ot[:], ps[:], bias_t[:, ts(n, N_CHUNK)]
                        )
                        nc.sync.dma_start(
                            out=out[ts(m, P), ts(n, N_CHUNK)], in_=ot[:]
                        )

    nc.compile()
    return nc


def _make_in_maps(x, W, b, lora_A, lora_B):
    xT = np.ascontiguousarray(x.reshape(TOK, D_IN).T)  # [D_IN, TOK]
    wT = np.ascontiguousarray(W.T)  # [D_IN, D_OUT]
    aT = np.ascontiguousarray(lora_A.T)  # [D_IN, R]
    bT = np.ascontiguousarray(SCALING * lora_B.T)  # [R, D_OUT]
    bias = np.ascontiguousarray(b[None, :])  # [1, D_OUT]
    return [
        {
            "xT": np.ascontiguousarray(xT[:, i * TOK_SHARD : (i + 1) * TOK_SHARD]),
            "wT": wT,
            "aT": aT,
            "bT": bT,
            "bias": bias,
        }
        for i in range(N_CORES)
    ]


def kernel(x, W, b, lora_A, lora_B):
    x = np.asarray(x, dtype=np.float32)
    W = np.asarray(W, dtype=np.float32)
    b = np.asarray(b, dtype=np.float32)
    lora_A = np.asarray(lora_A, dtype=np.float32)
    lora_B = np.asarray(lora_B, dtype=np.float32)

    if "main" not in _nc_cache:
        _nc_cache["main"] = _build()
    nc = _nc_cache["main"]

    in_maps = _make_in_maps(x, W, b, lora_A, lora_B)
    res = run_bass_kernel_spmd(nc, in_maps, list(range(N_CORES)))
    out = np.concatenate([res.results[i]["out"] for i in range(N_CORES)], axis=0)
    return out.reshape(B, S, D_OUT)
